# revision 1
# baseline (speedup 1.0000x reference)
"""Causal self-attention on 8 Trainium2 NeuronCores — v5 (fused phases).

Base design (v3, archived in kernel_v3.py) plus: projection work fused into
the attention loop as paced "extras" chunks; all projection
epilogues and xT copies on DVE (ACT does exp only); one shared PSUM
tag for all short-lived 2KB tiles (S double-buffer 4 banks + acc 1 +
shared 3 = 8). Out-projections are enqueued onto the extras queue only
AFTER the h2 division is emitted (deferral slot) - popping them earlier
read a stale attn tile (the v5.0 correctness bug, caught by CoreSim).
"""

import numpy as np

import concourse.bass as bass
import concourse.mybir as mybir
from concourse import bacc
from concourse import tile
from concourse.bass_utils import run_bass_kernel_spmd
from concourse.masks import make_identity

F32 = mybir.dt.float32
F32R = mybir.dt.float32r

EMBED = 768
NHEAD = 12
DH = 64
B = 2
T = 4096
HPC = 3
CH = HPC * DH
NCORES = 8
QW = 512


def build_program(t=T):
    nqb = t // QW

    nc = bacc.Bacc("TRN2", target_bir_lowering=False, debug=False,
                   num_devices=NCORES)

    x_d = nc.dram_tensor("x", [t, EMBED], F32, kind="ExternalInput")
    wqkv_d = nc.dram_tensor("wqkvT", [EMBED, 576], F32, kind="ExternalInput")
    bqkv_d = nc.dram_tensor("bqkv", [576, 1], F32, kind="ExternalInput")
    wo_d = nc.dram_tensor("woT", [CH, EMBED], F32, kind="ExternalInput")
    y_d = nc.dram_tensor("y", [t, EMBED], F32, kind="ExternalOutput")

    Act = mybir.ActivationFunctionType

    with tile.TileContext(nc) as tc:
        with (
            tc.tile_pool(name="const", bufs=1) as cpool,
            tc.tile_pool(name="persist", bufs=1) as perm,
        ):
            ident = cpool.tile([128, 128], F32, tag="ident")
            make_identity(nc, ident)
            identr = cpool.tile([128, 128], F32R, tag="identr")
            nc.vector.tensor_copy(identr, ident)
            ones_t = cpool.tile([128, 64], F32R, tag="ones")
            nc.gpsimd.memset(ones_t.bitcast(F32), 1.0)

            wqkv_sb = [cpool.tile([128, 576], F32R, name=f"wqkv{kt}",
                                  tag=f"wqkv{kt}") for kt in range(6)]
            wo_sb = [cpool.tile([64, EMBED], F32R, name=f"wo{h}",
                                tag=f"wo{h}") for h in range(3)]
            bias_sb = []
            for mc in range(5):
                mw = 128 if mc < 4 else 64
                b_t = cpool.tile([128, 1], F32, name=f"bias{mc}",
                                 tag=f"bias{mc}")
                nc.sync.dma_start(b_t[:mw, :],
                                  bqkv_d[mc * 128:mc * 128 + mw, :])
                bias_sb.append(b_t)
            with tc.tile_pool(name="wraw", bufs=1) as wraw:
                for kt in range(6):
                    w_raw = wraw.tile([128, 576], F32, name=f"wqkvraw{kt}",
                                      tag=f"wqkvraw{kt}")
                    nc.sync.dma_start(w_raw,
                                      wqkv_d[kt * 128:(kt + 1) * 128, :])
                    nc.vector.tensor_copy(wqkv_sb[kt], w_raw)
                for h in range(3):
                    wo_raw = wraw.tile([64, EMBED], F32, name=f"woraw{h}",
                                       tag=f"woraw{h}")
                    nc.sync.dma_start(wo_raw, wo_d[h * 64:(h + 1) * 64, :])
                    nc.vector.tensor_copy(wo_sb[h], wo_raw)

            q01 = perm.tile([128, t], F32R, tag="q01")
            k01 = perm.tile([128, t], F32R, tag="k01")
            qv0 = perm.tile([128, t], F32R, tag="qv0")
            kv1 = perm.tile([128, t], F32R, tag="kv1")
            v2t = perm.tile([64, t], F32R, tag="v2t")
            vs = [perm.tile([128, (t // 128) * 65], F32R, name=f"vs{h}",
                            tag=f"vs{h}")
                  for h in range(3)]
            for h in range(3):
                nc.gpsimd.memset(vs[h].bitcast(F32), 1.0)

            proj_dest = [q01, k01, qv0, kv1, v2t]

            def q_ap(h):
                return (q01[0:64], q01[64:128], qv0[0:64])[h]

            def k_ap(h):
                return (k01[0:64], k01[64:128], kv1[0:64])[h]

            v_src = [qv0[64:128], kv1[64:128], v2t[0:64]]
            v_idn = [identr[64:128, 64:128], identr[64:128, 64:128],
                     identr[0:64, 0:64]]

            import contextlib
            stack = contextlib.ExitStack()
            xpool = stack.enter_context(tc.tile_pool(name="xpool", bufs=4))
            xtpool = stack.enter_context(tc.tile_pool(name="xtpool", bufs=2))
            spsum = stack.enter_context(
                tc.tile_pool(name="spsum", bufs=2, space="PSUM"))
            accpsum = stack.enter_context(
                tc.tile_pool(name="accpsum", bufs=1, space="PSUM"))
            upsum = stack.enter_context(
                tc.tile_pool(name="upsum", bufs=3, space="PSUM"))
            ppool = stack.enter_context(tc.tile_pool(name="ppool", bufs=3))
            apool = stack.enter_context(tc.tile_pool(name="apool", bufs=2))
            rpool = stack.enter_context(tc.tile_pool(name="rpool", bufs=2))
            ysb = stack.enter_context(tc.tile_pool(name="ysb", bufs=2))

            def a_chunks(tb):
                xns = []
                xts = []

                def c_load():
                    for i in range(4):
                        row0 = tb * QW + i * 128
                        xn = xpool.tile([128, EMBED], F32, tag="xn",
                                        name=f"xn{tb}_{i}")
                        nc.sync.dma_start(xn, x_d[row0:row0 + 128, :])
                        xns.append(xn)
                    for ct in range(6):
                        xts.append(xtpool.tile(
                            [128, 512], F32R, tag=f"xt{ct}",
                            name=f"xt{ct}_{tb}"))

                def c_tr(ct):
                    def f():
                        tpg = upsum.tile([128, 512], F32, tag="u2k",
                                         name=f"tpg{tb}_{ct}")
                        for i in range(4):
                            nc.tensor.transpose(
                                tpg[:, i * 128:(i + 1) * 128],
                                xns[i][:, ct * 128:(ct + 1) * 128], ident)
                        nc.vector.tensor_copy(xts[ct], tpg)
                    return f

                def c_proj(mc):
                    def f():
                        mw = 128 if mc < 4 else 64
                        ps = upsum.tile([128, 512], F32, tag="u2k",
                                        name=f"proj{tb}_{mc}")
                        for ct in range(6):
                            nc.tensor.matmul(
                                ps[:mw, :],
                                lhsT=wqkv_sb[ct][:, mc * 128:mc * 128 + mw],
                                rhs=xts[ct],
                                start=(ct == 0), stop=(ct == 5))
                        dest = proj_dest[mc][:, tb * QW:(tb + 1) * QW]
                        nc.vector.tensor_scalar_add(dest, ps[:mw, :],
                                                    bias_sb[mc][:mw, :])
                    return f

                def c_vt(h):
                    def f():
                        vtile = upsum.tile([128, 512], F32R, tag="u2k",
                                           name=f"vt{h}_{tb}")
                        for i in range(4):
                            ck = tb * 4 + i
                            nc.tensor.transpose(
                                vtile[:, i * 64:(i + 1) * 64],
                                v_src[h][:, ck * 128:(ck + 1) * 128],
                                v_idn[h])
                        src = vtile[:, 0:256].rearrange(
                            "p (c w) -> p c w", w=64)
                        dst = vs[h].rearrange("p (c w) -> p c w", w=65)[
                            :, tb * 4:tb * 4 + 4, 0:64]
                        nc.vector.tensor_copy(dst, src)
                    return f

                chunks = [c_load]
                chunks += [c_tr(ct) for ct in range(6)]
                chunks += [c_proj(mc) for mc in range(5)]
                chunks += [c_vt(h) for h in range(3)]
                return chunks

            stages = []
            for qb in range(nqb):
                kbn = (qb + 1) * QW // 128
                ng = kbn // 2
                for h in range(3):
                    for g in range(ng):
                        stages.append((qb, h, g, 2 * g, g == ng - 1))
            nstages = len(stages)

            sp_t = {}
            pt_t = {}
            acc_t = {}
            rec_t = {}
            attn = {}
            deferred = {}

            def defer(slot, fn):
                deferred.setdefault(slot, []).append(fn)

            def emit_S(i):
                qb, h, g, kb0, last = stages[i]
                sp = spsum.tile([128, 2 * QW], F32, tag="s",
                                name=f"s{qb}_{h}_{g}")
                sp_t[i] = sp
                q_sl = slice(qb * QW, (qb + 1) * QW)
                for j in range(2):
                    kbi = kb0 + j
                    nc.tensor.matmul(
                        sp[:, j * QW:(j + 1) * QW],
                        lhsT=k_ap(h)[:, kbi * 128:(kbi + 1) * 128],
                        rhs=q_ap(h)[:, q_sl],
                        start=True, stop=True)

            def emit_exp_mask(i):
                qb, h, g, kb0, last = stages[i]
                kbn = (qb + 1) * QW // 128
                pt = ppool.tile([128, 2 * QW], F32R, tag="p",
                                name=f"p{qb}_{h}_{g}")
                pt_t[i] = pt
                nc.scalar.activation(pt, sp_t[i], Act.Exp,
                                     bias=0.0, scale=0.125)
                for j in range(2):
                    kbi = kb0 + j
                    if kbi >= kbn - 4:
                        v = pt[:, j * QW:(j + 1) * QW]
                        nc.gpsimd.affine_select(
                            out=v, in_=v,
                            compare_op=mybir.AluOpType.is_ge,
                            fill=0.0, base=qb * QW - kbi * 128,
                            pattern=[[1, QW]], channel_multiplier=-1)

            def emit_PV(i):
                qb, h, g, kb0, last = stages[i]
                if g == 0:
                    acc_t[(qb, h)] = accpsum.tile(
                        [65, QW], F32, tag="acc", name=f"acc{qb}_{h}")
                acc = acc_t[(qb, h)]
                kbn = (qb + 1) * QW // 128
                pt = pt_t.pop(i)
                for j in range(2):
                    kbi = kb0 + j
                    nc.tensor.matmul(
                        acc,
                        lhsT=vs[h][:, kbi * 65:kbi * 65 + 65],
                        rhs=pt[:, j * QW:(j + 1) * QW],
                        start=(kbi == 0), stop=(kbi == kbn - 1))
                sp_t.pop(i)

            def emit_recip(qb, h):
                acc = acc_t.pop((qb, h))
                accsb = rpool.tile([65, QW], F32, tag="accsb",
                                   name=f"accsb{qb}_{h}")
                nc.vector.tensor_copy(accsb, acc)
                rec = rpool.tile([65, QW], F32R, tag="rec",
                                 name=f"rec{qb}_{h}")
                rec_t[(qb, h)] = (accsb, rec)
                with nc.allow_low_precision(reason="fp32r rounding"):
                    nc.vector.reciprocal(rec[64:65], accsb[64:65])

            def emit_div(qb, h):
                accsb, rec = rec_t.pop((qb, h))
                bc = upsum.tile([128, 512], F32, tag="u2k",
                                name=f"bc{qb}_{h}")
                nc.tensor.matmul(bc[0:64, :], lhsT=ones_t[64:65, :],
                                 rhs=rec[64:65, :],
                                 start=True, stop=True)
                if h == 0:
                    attn[qb] = [apool.tile([64, QW], F32R, tag=f"attn{hh}",
                                           name=f"attn{hh}_{qb}")
                                for hh in range(3)]
                nc.vector.tensor_mul(attn[qb][h], accsb[0:64, :],
                                     bc[0:64, :])

            def emit_outproj(qb, mt):
                at = attn[qb]
                t_sl = slice(mt * 128, (mt + 1) * 128)
                row0 = qb * QW + mt * 128
                ys = ysb.tile([128, EMBED], F32, tag="ys",
                              name=f"ys{qb}_{mt}")
                for nh in range(2):
                    n_sl = slice(nh * 384, (nh + 1) * 384)
                    yp = upsum.tile([128, 512], F32, tag="u2k",
                                    name=f"y{qb}_{mt}_{nh}")
                    for h in range(3):
                        nc.tensor.matmul(yp[:, 0:384], lhsT=at[h][:, t_sl],
                                         rhs=wo_sb[h][:, n_sl],
                                         start=(h == 0), stop=(h == 2))
                    nc.vector.tensor_copy(ys[:, n_sl], yp[:, 0:384])
                nc.sync.dma_start(y_d[row0:row0 + 128, :], ys)
                if mt == 3:
                    attn.pop(qb)

            extras = []
            for fn in a_chunks(0):
                fn()
            extras.extend(a_chunks(1))

            emit_S(0)
            for i in range(nstages):
                qb, h, g, kb0, last = stages[i]
                if i + 1 < nstages:
                    if stages[i + 1][0] != qb:
                        while extras:
                            extras.pop(0)()
                        if qb + 2 < nqb:
                            extras.extend(a_chunks(qb + 2))
                    emit_S(i + 1)
                emit_exp_mask(i)
                for fn in deferred.pop(i, ()):
                    fn()
                emit_PV(i)
                if last:
                    emit_recip(qb, h)
                    defer(i + 1, lambda qb=qb, h=h: emit_div(qb, h))
                    if h == 2:
                        # enqueue out-proj only after div(qb, h2) has been
                        # emitted (same slot, deferred list runs in order)
                        defer(i + 1, lambda qb=qb: extras.extend(
                            lambda qb=qb, mt=mt: emit_outproj(qb, mt)
                            for mt in range(4)))
                if extras:
                    extras.pop(0)()
            for slot in sorted(deferred):
                for fn in deferred[slot]:
                    fn()
            while extras:
                extras.pop(0)()
            stack.close()
    nc.compile()
    return nc


_PROG_CACHE = {}


def _get_program(t=T):
    if t not in _PROG_CACHE:
        _PROG_CACHE[t] = build_program(t)
    return _PROG_CACHE[t]


def make_in_maps(x, wq, bq, wk, bk, wv, bv, wo):
    in_maps = []
    for core in range(NCORES):
        b = core // 4
        hs = (core % 4) * HPC
        sl = [slice((hs + h) * DH, (hs + h + 1) * DH) for h in range(HPC)]
        cols = [wq[sl[0]].T, wq[sl[1]].T, wk[sl[0]].T, wk[sl[1]].T,
                wq[sl[2]].T, wv[sl[0]].T, wk[sl[2]].T, wv[sl[1]].T,
                wv[sl[2]].T]
        biases = [bq[sl[0]], bq[sl[1]], bk[sl[0]], bk[sl[1]],
                  bq[sl[2]], bv[sl[0]], bk[sl[2]], bv[sl[1]], bv[sl[2]]]
        wqkvT = np.ascontiguousarray(np.concatenate(cols, axis=1),
                                     dtype=np.float32)
        bqkv = np.ascontiguousarray(
            np.concatenate(biases)[:, None], dtype=np.float32)
        ch = slice(hs * DH, (hs + HPC) * DH)
        woT = np.ascontiguousarray(wo[:, ch].T, dtype=np.float32)
        in_maps.append({
            "x": np.ascontiguousarray(x[b], dtype=np.float32),
            "wqkvT": wqkvT,
            "bqkv": bqkv,
            "woT": woT,
        })
    return in_maps


def run(inputs, t=T, trace=False, **kw):
    arrs = {k: np.asarray(v, dtype=np.float32) for k, v in inputs.items()}
    nc = _get_program(t)
    in_maps = make_in_maps(**arrs)
    res = run_bass_kernel_spmd(nc, in_maps, list(range(NCORES)),
                               trace=trace, **kw)
    outs = [np.asarray(m["y"], dtype=np.float32) for m in res.results]
    y = np.empty((B, t, EMBED), dtype=np.float32)
    for b in range(B):
        y[b] = outs[4 * b] + outs[4 * b + 1] + outs[4 * b + 2] + outs[4 * b + 3]
    return y, res


def kernel(**inputs):
    y, _ = run(inputs)
    return y



# revision 20
# speedup vs baseline: 1.5526x; 1.5526x over previous
"""Causal self-attention on 8 Trainium2 NeuronCores — v6.

Design (vs v5 baseline):
- All matmul operands bf16 (fp32 PSUM accumulation); DMA traffic halved,
  y written as bf16 partials and summed on host in fp32.
- x is transposed on the host: xT [768, t] is DMA'd straight into the
  resident SBUF tiles the projections consume — no PE transposes or DVE
  copies for x.
- S = K^T Q matmuls are K=64; pairs of them run concurrently on disjoint
  PE row-groups via tile_position (h0 rows 0-63 || h1 rows 64-127; h2 is
  paired with itself via duplicated Wq2/Wk2 projection columns so its
  kbi-pairs land on both row halves).
- Causal diagonal blocks are trimmed: S/exp/PV operate on per-kbi
  rectangles [off:512] (off = 128*(kbi-4qb)), and only the leading
  128x128 triangle of each diagonal block is masked on gpsimd.
- Attention core otherwise follows v5: exp on ACT (scale=1/8), softmax
  denominator via an extra all-ones column in the V tiles, reciprocal +
  PE-broadcast + DVE multiply for normalization, head-stacked out-proj
  accumulated in PSUM.
"""

import contextlib

import numpy as np
import ml_dtypes

import concourse.bass as bass
import concourse.mybir as mybir
from concourse import bacc
from concourse import tile
from concourse.bass_utils import run_bass_kernel_spmd
from concourse.masks import make_identity

F32 = mybir.dt.float32
F32R = mybir.dt.float32r
BF16 = mybir.dt.bfloat16

EMBED = 768
NHEAD = 12
DH = 64
B = 2
T = 4096
HPC = 3
NCORES = 8
QW = 512
WCOLS = 704  # 5.5 mc blocks: q01 k01 q22 k22 v01 v2

Act = mybir.ActivationFunctionType

import os
USE_TILE_POSITION = os.environ.get("K_NO_TILEPOS", "") == ""
BF16_VT = os.environ.get("K_F32_VT", "") == ""
EAGER_X = os.environ.get("K_EAGER_X", "") != ""
# K_TRIM: "pv" (default) = full-width S/exp, trimmed PV matmuls;
# "all" = also trim S/exp rects (crashes HW at t>=1536 — cause unknown);
# "none" = full-width everything
TRIM = os.environ.get("K_TRIM", "pv")
if os.environ.get("K_NO_TRIM", "") != "":
    TRIM = "none"
NO_TRIM = TRIM == "none"


def _tp(pos):
    return pos if USE_TILE_POSITION else None


def build_program(t=T):
    nqb = t // QW
    nkb = t // 128

    nc = bacc.Bacc("TRN2", target_bir_lowering=False, debug=False,
                   num_devices=NCORES)

    xT_d = nc.dram_tensor("xT", [EMBED, t], BF16, kind="ExternalInput")
    wqkv_d = nc.dram_tensor("wqkvT", [EMBED, WCOLS], BF16,
                            kind="ExternalInput")
    bqkv_d = nc.dram_tensor("bqkv", [WCOLS, 1], F32, kind="ExternalInput")
    wo_d = nc.dram_tensor("woT", [HPC * DH, EMBED], BF16,
                          kind="ExternalInput")
    y_d = nc.dram_tensor("y", [t, EMBED], BF16, kind="ExternalOutput")

    with tile.TileContext(nc) as tc:
        with (
            tc.tile_pool(name="const", bufs=1) as cpool,
            tc.tile_pool(name="persist", bufs=1) as perm,
        ):
            ident = cpool.tile([128, 128], BF16, tag="ident")
            make_identity(nc, ident)
            if not BF16_VT:
                identf = cpool.tile([128, 128], F32, tag="identf")
                make_identity(nc, identf)
            ones_t = cpool.tile([128, 64], F32R, tag="ones")
            nc.gpsimd.memset(ones_t.bitcast(F32), 1.0)

            wq_sb = [cpool.tile([128, WCOLS], BF16, name=f"wq{ct}",
                                tag=f"wq{ct}") for ct in range(6)]
            for ct in range(6):
                nc.sync.dma_start(wq_sb[ct],
                                  wqkv_d[ct * 128:(ct + 1) * 128, :])
            wo_sb = [cpool.tile([64, EMBED], BF16, name=f"wo{h}",
                                tag=f"wo{h}") for h in range(3)]
            for h in range(3):
                nc.sync.dma_start(wo_sb[h], wo_d[h * 64:(h + 1) * 64, :])
            bias_sb = []
            for mc in range(6):
                mw = 128 if mc < 5 else 64
                b_t = cpool.tile([128, 1], F32, name=f"bias{mc}",
                                 tag=f"bias{mc}")
                nc.sync.dma_start(b_t[:mw, :],
                                  bqkv_d[mc * 128:mc * 128 + mw, :])
                bias_sb.append(b_t)

            # resident xT tiles, DMA'd per-tb slices
            xt = [perm.tile([128, t], BF16, name=f"xt{ct}", tag=f"xt{ct}")
                  for ct in range(6)]
            q01 = perm.tile([128, t], BF16, tag="q01")
            k01 = perm.tile([128, t], BF16, tag="k01")
            q22 = perm.tile([128, t], BF16, tag="q22")
            k22 = perm.tile([128, t], BF16, tag="k22")
            vs = [perm.tile([128, nkb * 65], BF16, name=f"vs{h}",
                            tag=f"vs{h}") for h in range(3)]
            for h in range(3):
                nc.gpsimd.memset(vs[h], 1.0)

            qk_dest = [q01, k01, q22, k22]

            stack = contextlib.ExitStack()
            spsum = stack.enter_context(
                tc.tile_pool(name="spsum", bufs=2, space="PSUM"))
            accpsum = stack.enter_context(
                tc.tile_pool(name="accpsum", bufs=2, space="PSUM"))
            upsum = stack.enter_context(
                tc.tile_pool(name="upsum", bufs=2, space="PSUM"))
            ppool = stack.enter_context(tc.tile_pool(name="ppool", bufs=3))
            vtpool = stack.enter_context(tc.tile_pool(name="vtpool", bufs=2))
            rpool = stack.enter_context(tc.tile_pool(name="rpool", bufs=2))
            apool = stack.enter_context(tc.tile_pool(name="apool", bufs=2))
            ysb = stack.enter_context(tc.tile_pool(name="ysb", bufs=2))

            # ---------------- projection extras (per 512-row tb) --------
            def a_chunks(tb):
                tb_sl = slice(tb * QW, (tb + 1) * QW)
                vtmp = {}

                def c_xload():
                    if EAGER_X and tb > 0:
                        return
                    for ct in range(6):
                        sl = slice(None) if EAGER_X else tb_sl
                        nc.sync.dma_start(
                            xt[ct][:, sl],
                            xT_d[ct * 128:(ct + 1) * 128, sl])

                def c_proj(mc):
                    def f():
                        mw = 128 if mc < 5 else 64
                        c0 = mc * 128
                        ps = upsum.tile([128, QW], F32, tag="u",
                                        name=f"proj{tb}_{mc}")
                        for ct in range(6):
                            nc.tensor.matmul(
                                ps[:mw, :],
                                lhsT=wq_sb[ct][:, c0:c0 + mw],
                                rhs=xt[ct][:, tb_sl],
                                start=(ct == 0), stop=(ct == 5))
                        if mc < 4:
                            dest = qk_dest[mc][:, tb_sl]
                            nc.vector.tensor_scalar_add(
                                dest, ps[:mw, :], bias_sb[mc][:mw, :])
                        else:
                            vt = vtpool.tile([128, QW], BF16 if BF16_VT else F32,
                                             tag=f"vtmp{mc}",
                                             name=f"vtmp{tb}_{mc}")
                            vtmp[mc] = vt
                            nc.vector.tensor_scalar_add(
                                vt[:mw, :], ps[:mw, :], bias_sb[mc][:mw, :])
                    return f

                def c_vt(h):
                    def f():
                        src = (vtmp[4][0:64], vtmp[4][64:128],
                               vtmp[5][0:64])[h]
                        idt = ident if BF16_VT else identf
                        idn = (idt[0:64, 0:64], idt[64:128, 64:128],
                               idt[0:64, 0:64])[h]
                        vtile = upsum.tile([128, 2 * QW] if BF16_VT else [128, QW], BF16 if BF16_VT else F32, tag="u",
                                           name=f"vt{h}_{tb}")
                        for i in range(4):
                            nc.tensor.transpose(
                                vtile[:, i * 64:(i + 1) * 64],
                                src[:, i * 128:(i + 1) * 128],
                                idn)
                        s2 = vtile[:, 0:256].rearrange(
                            "p (c w) -> p c w", w=64)
                        dst = vs[h].rearrange("p (c w) -> p c w", w=65)[
                            :, tb * 4:tb * 4 + 4, 0:64]
                        nc.vector.tensor_copy(dst, s2)
                    return f

                chunks = [c_xload]
                chunks += [c_proj(mc) for mc in range(6)]
                chunks += [c_vt(h) for h in range(3)]
                return chunks

            # ---------------- attention stages ---------------------------
            # group list: per qb, phase A (h0&h1 row-tiled pairs), then
            # phase B (h2 paired with itself via duplicated K/Q columns).
            groups = []
            for qb in range(nqb):
                ng = (qb + 1) * 2
                for g in range(ng):
                    groups.append((qb, "A", g, g == ng - 1))
                for g in range(ng):
                    groups.append((qb, "B", g, g == ng - 1))
            ngroups = len(groups)

            def rects(qb, g):
                """(kbi, in-tile col offset, width) for the kbi pair."""
                out = []
                rs = 0
                for kbi in (2 * g, 2 * g + 1):
                    off = (max(0, (kbi - 4 * qb) * 128)
                           if TRIM == "all" else 0)
                    w = QW - off
                    out.append((kbi, rs, off, w))
                    rs += w
                return out

            sp_t = {}
            pt_t = {}
            acc_t = {}
            rec_t = {}
            attn = {}
            deferred = {}

            def defer(slot, fn):
                deferred.setdefault(slot, []).append(fn)

            def emit_S(i):
                qb, ph, g, last = groups[i]
                q_base = qb * QW
                if ph == "A":
                    sA = spsum.tile([128, 2 * QW], F32, tag="s",
                                    name=f"sA{qb}_{g}")
                    sB = spsum.tile([128, 2 * QW], F32, tag="s",
                                    name=f"sB{qb}_{g}")
                    sp_t[i] = (sA, sB)
                    for kbi, rs, off, w in rects(qb, g):
                        k_sl = slice(kbi * 128, (kbi + 1) * 128)
                        q_sl = slice(q_base + off, q_base + QW)
                        nc.tensor.matmul(
                            sA[:, rs:rs + w],
                            lhsT=k01[0:64, k_sl], rhs=q01[0:64, q_sl],
                            start=True, stop=True, tile_position=_tp((0, 0)))
                        nc.tensor.matmul(
                            sB[:, rs:rs + w],
                            lhsT=k01[64:128, k_sl], rhs=q01[64:128, q_sl],
                            start=True, stop=True, tile_position=_tp((64, 0)))
                else:
                    sC = spsum.tile([128, 2 * QW], F32, tag="s",
                                    name=f"sC{qb}_{g}")
                    sp_t[i] = (sC,)
                    for idx, (kbi, rs, off, w) in enumerate(rects(qb, g)):
                        k_sl = slice(kbi * 128, (kbi + 1) * 128)
                        q_sl = slice(q_base + off, q_base + QW)
                        r0 = idx * 64
                        nc.tensor.matmul(
                            sC[:, rs:rs + w],
                            lhsT=k22[r0:r0 + 64, k_sl],
                            rhs=q22[r0:r0 + 64, q_sl],
                            start=True, stop=True, tile_position=_tp((r0, 0)))

            def emit_exp_mask(i):
                qb, ph, g, last = groups[i]
                rcs = rects(qb, g)
                wsum = sum(r[3] for r in rcs)
                hs = (0, 1) if ph == "A" else (2,)
                pts = []
                for hi, h in enumerate(hs):
                    sp = sp_t[i][hi]
                    pt = ppool.tile([128, 2 * QW], BF16, tag="p",
                                    name=f"p{qb}_{ph}{g}_{h}")
                    pts.append(pt)
                    nc.scalar.activation(pt[:, 0:wsum], sp[:, 0:wsum],
                                         Act.Exp, bias=0.0, scale=0.125)
                    for kbi, rs, off, w in rcs:
                        if kbi >= 4 * qb:
                            if TRIM == "none":
                                # PV reads the full rect: mask everything
                                # left of the diagonal too
                                v = pt[:, rs:rs + QW]
                                nc.gpsimd.affine_select(
                                    out=v, in_=v,
                                    compare_op=mybir.AluOpType.is_ge,
                                    fill=0.0, base=qb * QW - kbi * 128,
                                    pattern=[[1, QW]],
                                    channel_multiplier=-1)
                            else:
                                # PV starts reading at the diagonal block:
                                # only its 128x128 triangle needs masking
                                moff = 0 if TRIM == "all" else (
                                    (kbi - 4 * qb) * 128)
                                v = pt[:, rs + moff:rs + moff + 128]
                                nc.gpsimd.affine_select(
                                    out=v, in_=v,
                                    compare_op=mybir.AluOpType.is_ge,
                                    fill=0.0, base=0,
                                    pattern=[[1, 128]],
                                    channel_multiplier=-1)
                pt_t[i] = pts

            def emit_PV(i):
                qb, ph, g, last = groups[i]
                kbn = 4 * (qb + 1)
                hs = (0, 1) if ph == "A" else (2,)
                pts = pt_t.pop(i)
                for hi, h in enumerate(hs):
                    if g == 0:
                        acc_t[(qb, h)] = accpsum.tile(
                            [65, QW], F32, tag="acc", name=f"acc{qb}_{h}")
                    acc = acc_t[(qb, h)]
                    pt = pts[hi]
                    for kbi, rs, off, w in rects(qb, g):
                        if TRIM == "pv":
                            off = max(0, (kbi - 4 * qb) * 128)
                            rs = rs + off
                        nc.tensor.matmul(
                            acc[:, off:QW],
                            lhsT=vs[h][:, kbi * 65:kbi * 65 + 65],
                            rhs=pt[:, rs:rs + (QW - off)],
                            start=(kbi == 0), stop=(kbi == kbn - 1))
                sp_t.pop(i)

            def emit_recip(qb, h):
                acc = acc_t.pop((qb, h))
                accsb = rpool.tile([65, QW], F32, tag="accsb",
                                   name=f"accsb{qb}_{h}")
                nc.vector.tensor_copy(accsb, acc)
                rec = rpool.tile([65, QW], F32R, tag="rec",
                                 name=f"rec{qb}_{h}")
                rec_t[(qb, h)] = (accsb, rec)
                with nc.allow_low_precision(reason="fp32r rounding"):
                    nc.vector.reciprocal(rec[64:65], accsb[64:65])

            def emit_div(qb, h):
                accsb, rec = rec_t.pop((qb, h))
                bc = upsum.tile([128, QW], F32, tag="u",
                                name=f"bc{qb}_{h}")
                nc.tensor.matmul(bc[0:64, :], lhsT=ones_t[64:65, :],
                                 rhs=rec[64:65, :],
                                 start=True, stop=True)
                if h == 0:
                    attn[qb] = [apool.tile([64, QW], BF16, tag=f"attn{hh}",
                                           name=f"attn{hh}_{qb}")
                                for hh in range(3)]
                nc.vector.tensor_mul(attn[qb][h], accsb[0:64, :],
                                     bc[0:64, :])

            def emit_outproj(qb, mt):
                at = attn[qb]
                t_sl = slice(mt * 128, (mt + 1) * 128)
                row0 = qb * QW + mt * 128
                ys = ysb.tile([128, EMBED], BF16, tag="ys",
                              name=f"ys{qb}_{mt}")
                for nh in range(2):
                    n_sl = slice(nh * 384, (nh + 1) * 384)
                    yp = upsum.tile([128, QW], F32, tag="u",
                                    name=f"y{qb}_{mt}_{nh}")
                    for h in range(3):
                        nc.tensor.matmul(yp[:, 0:384], lhsT=at[h][:, t_sl],
                                         rhs=wo_sb[h][:, n_sl],
                                         start=(h == 0), stop=(h == 2))
                    nc.vector.tensor_copy(ys[:, n_sl], yp[:, 0:384])
                nc.sync.dma_start(y_d[row0:row0 + 128, :], ys)
                if mt == 3:
                    attn.pop(qb)

            # ---------------- main emission loop -------------------------
            extras = []
            for fn in a_chunks(0):
                fn()
            extras.extend(a_chunks(1))

            emit_S(0)
            for i in range(ngroups):
                qb, ph, g, last = groups[i]
                if i + 1 < ngroups:
                    if groups[i + 1][0] != qb:
                        while extras:
                            extras.pop(0)()
                        if qb + 2 < nqb:
                            extras.extend(a_chunks(qb + 2))
                    emit_S(i + 1)
                emit_exp_mask(i)
                for fn in deferred.pop(i, ()):
                    fn()
                emit_PV(i)
                if last:
                    if ph == "A":
                        emit_recip(qb, 0)
                        emit_recip(qb, 1)
                        defer(i + 1, lambda qb=qb: emit_div(qb, 0))
                        defer(i + 1, lambda qb=qb: emit_div(qb, 1))
                    else:
                        emit_recip(qb, 2)
                        defer(i + 1, lambda qb=qb: emit_div(qb, 2))
                        defer(i + 1, lambda qb=qb: extras.extend(
                            lambda qb=qb, mt=mt: emit_outproj(qb, mt)
                            for mt in range(4)))
                if extras:
                    extras.pop(0)()
            for slot in sorted(deferred):
                for fn in deferred[slot]:
                    fn()
            while extras:
                extras.pop(0)()
            stack.close()
    nc.compile()
    return nc


_PROG_CACHE = {}


def _get_program(t=T):
    if t not in _PROG_CACHE:
        _PROG_CACHE[t] = build_program(t)
    return _PROG_CACHE[t]


def _bf16(a):
    return np.ascontiguousarray(np.asarray(a, np.float32)).astype(
        ml_dtypes.bfloat16)


def make_in_maps(x, wq, bq, wk, bk, wv, bv, wo, t=T):
    in_maps = []
    for core in range(NCORES):
        b = core // 4
        hs = (core % 4) * HPC
        sl = [slice((hs + h) * DH, (hs + h + 1) * DH) for h in range(HPC)]
        wcols = [wq[sl[0]].T, wq[sl[1]].T,
                 wk[sl[0]].T, wk[sl[1]].T,
                 wq[sl[2]].T, wq[sl[2]].T,
                 wk[sl[2]].T, wk[sl[2]].T,
                 wv[sl[0]].T, wv[sl[1]].T,
                 wv[sl[2]].T]
        biases = [bq[sl[0]], bq[sl[1]], bk[sl[0]], bk[sl[1]],
                  bq[sl[2]], bq[sl[2]], bk[sl[2]], bk[sl[2]],
                  bv[sl[0]], bv[sl[1]], bv[sl[2]]]
        wqkvT = _bf16(np.concatenate(wcols, axis=1))
        bqkv = np.ascontiguousarray(
            np.concatenate(biases)[:, None], dtype=np.float32)
        ch = slice(hs * DH, (hs + HPC) * DH)
        woT = _bf16(wo[:, ch].T)
        in_maps.append({
            "xT": _bf16(np.asarray(x[b][:t], np.float32).T),
            "wqkvT": wqkvT,
            "bqkv": bqkv,
            "woT": woT,
        })
    return in_maps


def run(inputs, t=T, trace=False, **kw):
    arrs = {k: np.asarray(v, dtype=np.float32) for k, v in inputs.items()}
    nc = _get_program(t)
    in_maps = make_in_maps(**arrs, t=t)
    res = run_bass_kernel_spmd(nc, in_maps, list(range(NCORES)),
                               trace=trace, **kw)
    outs = [np.asarray(m["y"], dtype=np.float32) for m in res.results]
    y = np.empty((B, t, EMBED), dtype=np.float32)
    for b in range(B):
        y[b] = outs[4 * b] + outs[4 * b + 1] + outs[4 * b + 2] + outs[4 * b + 3]
    return y, res


def kernel(**inputs):
    y, _ = run(inputs)
    return y
